# revision 12
# baseline (speedup 1.0000x reference)
"""Trainium2 Bass kernel for CausalAttentionSortNet bucket-scoring.

Math (see reference): only `k` feeds the output. For each merged batch*head
slice, the cumulative-average of k is sampled at bucket starts (every 128th
row), which reduces to per-chunk sums + a strictly-triangular prefix matmul.
The rest is tiny per-bucket sort projections and a 64x65 masked softmax.

Sharding: data-parallel over the merged (batch*heads)=32 axis across 8 cores,
4 slices per core, processed as 2 pairs of 2 slices; a pair fills the
128-partition dim as partition=(slice_in_pair, chunk), free=(pair, row, dim)
so every partition's k data is contiguous 32KB HBM runs (DMA saturates all
16 engines at ~350 GB/s).

`q` (half of all input bytes) is never read by the reference computation, so
it is not even transferred to the device.

DMA-instruction budget: the hardware exposes ~12 DMA completion semaphores,
so a DMA instruction >=12 positions later reuses an earlier one's semaphore
and its issue blocks until that user completes. All constants ship in two
early-completing DMAs and the bulk tiles are uniform, so every reuse target
is long done by the time its semaphore is recycled (a version that put a
slow small-packet constant DMA in the reuse chain stalled the bulk queue
12us). Chunk first-rows are not a separate DMA: they arrive inside each
pair's first bulk tile, whose fold targets the tile's upper half so row 0
survives for the F-term matmuls.

Per-chunk reduction: each pair's rows stream as sub-tiles of
(16x7, 8, 4, 4) rows. Mid-stream, SBUF port contention caps DVE at
~1.8ns/elem and GpSimd at ~2.6ns/elem (vs 1.04/2.0 idle), so each sub-tile
gets an INDEPENDENT halving-fold chain (contiguous tensor_adds down to one
row -> its own partial-sum slot) and the chains are statically balanced
across both engines; a long serial cascade on one engine trailed the
stream by 12us. The PE (otherwise idle) folds every partial into the
scaled-prefix via one matmul per sub-tile against the tril*scale constant,
accumulating in that pair's PSUM bank, opened by the F*diag(s) seed and
closed by the last sub-tile's matmul. Small sub-tiles stream last so the
post-stream tail is two ~0.3us fold chains plus the epilogue.
"""

from contextlib import ExitStack

import numpy as np

import concourse.bacc as bacc
import concourse.mybir as mybir
import concourse.tile as tile
from concourse import bass_utils

# Problem constants (hardcoded per contract; kernel.py must be self-contained).
B, HEADS, BUCKETS, DIM, DIM_SORT, T = 4, 8, 64, 64, 8, 8192
BH = B * HEADS            # 32 merged batch*head slices
NCORES = 8
BHC = BH // NCORES        # 4 slices per core
NPAIR = BHC // 2          # 2 pairs per core
CHUNK = T // BUCKETS      # 128 rows per bucket
NEG = -1.0e30             # softmax mask value (underflows exp to exactly 0)
FP = mybir.dt.float32

# packed-constant column offsets
NC128 = 128 * 3 + 136 * 2
NC64 = 4 * 104
NC104 = 2 * 128
NCALL = NC128 + NC104

# pair-1 fold chains for sub-tiles [0, GP_CHAINS) run on GpSimd; all other
# chains (including every chain near the tail) on the faster DVE
GP_CHAINS = 7

TRACE = False  # set by test.py for profiling runs
TRACE_KWARGS = {}  # extra run_bass_kernel_spmd kwargs for profiling runs
LAST_RESULTS = None  # BassKernelResults of the most recent run

_PROG_CACHE = {}


def _cascade_sizes(chunk):
    # uniform mid-size tiles, small ones last: (16x7, 8, 4, 4) for chunk=128
    assert chunk == 128, "sub-tile schedule is tuned for chunk=128"
    sizes = [16] * 7 + [8, 4, 4]
    assert sum(sizes) == chunk, (sizes, chunk)
    return sizes


def _build_program(t_seq=T, enable_asserts=False, debug_taps=False):
    chunk = t_seq // BUCKETS
    sizes = _cascade_sizes(chunk)
    nsub = len(sizes)

    nc = bacc.Bacc(
        "TRN2",
        target_bir_lowering=False,
        debug=False,
        enable_asserts=enable_asserts,
        num_devices=NCORES,
    )

    def din(name, shape):
        return nc.dram_tensor(name, shape, FP, kind="ExternalInput").ap()

    kin = din("kin", (BHC, t_seq, DIM))
    # packed constants, two DMAs:
    # cpack cols 0:656     [lmat_s | idents | ident | am68 | mm68]
    # cpack cols 644:900   c104 = per pair (104, 128) cq/ck blocks (rows 104:128 zero)
    # c64 (64, 416)        [wqk_pt_p0 | wqk_pt_p1 | wqk_ft_p0 | wqk_ft_p1]
    cpack = din("cpack", (128, NCALL))
    c64 = din("c64", (64, NC64))
    rout = nc.dram_tensor(
        "rout", (BHC, BUCKETS, BUCKETS + 1), FP, kind="ExternalOutput"
    ).ap()
    taps = {}
    if debug_taps:
        taps["pt"] = nc.dram_tensor("tap_pt", (128, 128), FP, kind="ExternalOutput").ap()
        taps["ft"] = nc.dram_tensor("tap_ft", (128, 128), FP, kind="ExternalOutput").ap()
        taps["par"] = nc.dram_tensor("tap_par", (128, 128), FP, kind="ExternalOutput").ap()

    X = mybir.AxisListType.X
    Exp = mybir.ActivationFunctionType.Exp
    MULT = mybir.AluOpType.mult

    with tile.TileContext(nc) as tc:
        with ExitStack() as ctx:
            singles = ctx.enter_context(tc.tile_pool(name="singles", bufs=1))
            kpools = [
                ctx.enter_context(tc.tile_pool(name=f"kpool{s}", bufs=2))
                for s in range(nsub)
            ]
            parp = ctx.enter_context(tc.tile_pool(name="parp", bufs=nsub))
            small = ctx.enter_context(tc.tile_pool(name="small", bufs=2))
            pp = ctx.enter_context(tc.tile_pool(name="pp", bufs=1, space="PSUM"))

            cp_sb = singles.tile([128, NCALL], FP, tag="cpack")
            nc.scalar.dma_start(cp_sb[:], cpack)
            c64_sb = singles.tile([64, NC64], FP, tag="c64")
            nc.scalar.dma_start(c64_sb[:], c64)

            # ---- bulk k sub-tile DMAs, single queue, pair 1 leading so
            # its GpSimd chains start first (contiguous rows*256B runs per
            # partition)
            ksrcs = [
                kin[2 * p : 2 * p + 2].rearrange("b (c r) d -> (b c) r d", r=chunk)
                for p in range(NPAIR)
            ]
            kts = {}
            r0 = 0
            for s, rs in enumerate(sizes):
                for p in (1, 0):
                    kt = kpools[s].tile(
                        [128, rs, DIM], FP, tag=f"kt{s}", name=f"kt{s}_{p}"
                    )
                    nc.sync.dma_start(kt[:], ksrcs[p][:, r0 : r0 + rs, :])
                    kts[(p, s)] = kt
                r0 += rs

            lmat_s = cp_sb[:, 0:128]
            idents = cp_sb[:, 128:256]
            ident = cp_sb[:, 256:384]
            # am68 (cols 384:520) is consumed directly by the R-group matmul
            mm68 = cp_sb[:, 520:656].rearrange("q (p j) -> q p j", p=2)

            # ---- PSUM groups, one bank per (pair, tensor): FT_p is F
            # transposed; PT_p is opened by the F*diag(s) seed and closed by
            # that pair's chunk-sum prefix matmul. F = row 0 of the first
            # bulk tile. Separate banks let pair 0's whole epilogue run while
            # pair 1 is still streaming.
            # full-partition PSUM tiles: a 64-partition tile can be packed
            # at partition offset 64 of another tile's bank, and its group's
            # deferred-zero bookkeeping then poisons that tile's rows 64:128
            PT_ps = [
                pp.tile([128, 128], FP, tag=f"PT{p}", name=f"PT_ps{p}")
                for p in range(NPAIR)
            ]
            FT_ps = [
                pp.tile([128, 128], FP, tag=f"FT{p}", name=f"FT_ps{p}")
                for p in range(NPAIR)
            ]
            for p in range(NPAIR):
                nc.tensor.matmul(
                    FT_ps[p][0:64, :],
                    lhsT=kts[(p, 0)][:, 0, :],
                    rhs=ident,
                    start=True,
                    stop=True,
                )
                nc.tensor.matmul(
                    PT_ps[p][0:64, :],
                    lhsT=kts[(p, 0)][:, 0, :],
                    rhs=idents,
                    start=True,
                    stop=False,
                )

            # ---- per-sub-tile fold chains + per-sub-tile prefix matmuls.
            # Each (pair, sub-tile) folds independently down to one row (the
            # first fold targets the upper half so row 0 survives in tile 0),
            # writing its own partial-sum slot; the PE folds every partial
            # into the pair's scaled-prefix PSUM bank as it appears.
            pars = [
                parp.tile([128, NPAIR, DIM], FP, tag=f"par{s}", name=f"par{s}")
                for s in range(nsub)
            ]
            for s, rs in enumerate(sizes):
                for p in (1, 0):
                    t = kts[(p, s)]
                    e = nc.gpsimd if (p == 1 and s < GP_CHAINS) else nc.vector
                    h = rs // 2
                    e.tensor_add(t[:, h:rs, :], t[:, h:rs, :], t[:, 0:h, :])
                    lo, xr = h, h
                    while xr > 2:
                        hh = xr // 2
                        e.tensor_add(
                            t[:, lo : lo + hh, :],
                            t[:, lo : lo + hh, :],
                            t[:, lo + hh : lo + xr, :],
                        )
                        xr = hh
                    e.tensor_add(
                        pars[s][:, p, :], t[:, lo, :], t[:, lo + 1, :]
                    )
                    nc.tensor.matmul(
                        PT_ps[p][0:64, :],
                        lhsT=pars[s][:, p, :],
                        rhs=lmat_s,
                        start=False,
                        stop=s == nsub - 1,
                    )

            # ---- sort projections (per pair), batched softmax (both pairs)
            PT_sb = [
                small.tile([64, 128], FP, tag=f"PTs{p}", name=f"PT_sb{p}")
                for p in range(NPAIR)
            ]
            FT_sb = [
                small.tile([64, 128], FP, tag=f"FTs{p}", name=f"FT_sb{p}")
                for p in range(NPAIR)
            ]
            for p in range(NPAIR):
                nc.scalar.copy(FT_sb[p][:], FT_ps[p][0:64, :])
                nc.scalar.copy(PT_sb[p][:], PT_ps[p][0:64, :])
            if debug_taps:
                for p in range(NPAIR):
                    nc.sync.dma_start(taps["pt"][64 * p : 64 * p + 64], PT_sb[p][:])
                    nc.sync.dma_start(taps["ft"][64 * p : 64 * p + 64], FT_sb[p][:])


            # SKQ rows: 0:40 sort-q blocks (b0 at 0:8, b1 at 32:40),
            #           64:104 sort-k blocks (b0 at 64:72, b1 at 96:104);
            # one PSUM bank per pair: each holds a long-open accumulation group
            # opened by the constant-term matmul (ready at kernel start) and
            # closed by the PT-part matmul (the only one on the critical tail)
            C104O = NC128
            SQs = []
            RKs = []
            for p in range(NPAIR):
                sk_ps_t = pp.tile([128, 128], FP, tag=f"SKQ{p}", name=f"skq{p}")
                sk_ps = sk_ps_t[0:104, :]
                nc.tensor.matmul(
                    sk_ps,
                    lhsT=ident[0:104, 0:104],
                    rhs=cp_sb[0:104, C104O + 128 * p : C104O + 128 * p + 128],
                    start=True,
                    stop=False,
                    skip_group_check=True,
                )
                nc.tensor.matmul(
                    sk_ps,
                    lhsT=c64_sb[:, 208 + 104 * p : 312 + 104 * p],
                    rhs=FT_sb[p][:],
                    start=False,
                    stop=False,
                    skip_group_check=True,
                )
                nc.tensor.matmul(
                    sk_ps,
                    lhsT=c64_sb[:, 104 * p : 104 * p + 104],
                    rhs=PT_sb[p][:],
                    start=False,
                    stop=True,
                    skip_group_check=True,
                )
                sq_sb = small.tile([40, 128], FP, tag=f"SQ{p}")
                nc.scalar.copy(sq_sb[:], sk_ps[0:40, :])
                rk_sb = small.tile([40, 128], FP, tag=f"RK{p}")
                nc.vector.tensor_copy(rk_sb[:], sk_ps[64:104, :])
                SQs.append(sq_sb)
                RKs.append(rk_sb)

            # R group, 65 logit columns per pair: opened early by an
            # identity-weighted matmul that seeds the bank with the additive
            # causal mask (whose column 0 is the pad-row's constant zero
            # logit); the four sq.sk matmuls then accumulate into columns
            # 1:65 of their quadrants, so the masked logits sit in PSUM with
            # no extra elementwise pass and no separate zero-column handling
            # per-pair 68-wide blocks: col 0 pad, col 1 the pad-row's
            # constant zero logit (both written only by the mask seed), cols
            # 2:66 the sq.sk logits, 66:68 pad. The 68 width keeps the
            # partition-64 quadrant writes inside the seed-cleared PSUM
            # zero-region window (64 * width-bytes must be 0 mod 2048) and
            # their byte offsets 8-aligned.
            R_ps = pp.tile([128, 2 * 68], FP, tag="R")
            nc.tensor.matmul(
                R_ps[:],
                lhsT=ident,
                rhs=cp_sb[:, 384:520],
                start=True,
                stop=False,
                skip_group_check=True,
            )
            for p in range(NPAIR):
                nc.tensor.matmul(
                    R_ps[0:64, 68 * p + 2 : 68 * p + 66],
                    lhsT=SQs[p][0:8, 0:64],
                    rhs=RKs[p][0:8, 0:64],
                    start=False,
                    stop=False,
                    skip_group_check=True,
                )
                nc.tensor.matmul(
                    R_ps[64:128, 68 * p + 2 : 68 * p + 66],
                    lhsT=SQs[p][32:40, 64:128],
                    rhs=RKs[p][32:40, 64:128],
                    start=False,
                    stop=p == NPAIR - 1,
                    skip_group_check=True,
                )

            # masked softmax over the 65 logits, both pairs batched along
            # the free axis: cols = (pair, j). Column 0 (the zero logit) is
            # already in R, so max/sum/mask need no special-casing.
            Rm = R_ps[:].rearrange("q (p j) -> q p j", p=2)
            mx = small.tile([128, 2], FP, tag="mx")
            nc.vector.reduce_max(mx[:], Rm, axis=X)
            negm = small.tile([128, 2], FP, tag="negm")
            nc.vector.tensor_scalar(
                negm[:], mx[:], 0.0, -1.0,
                op0=mybir.AluOpType.max, op1=MULT,
            )
            e_sb = small.tile([128, 2, 68], FP, tag="e")
            for p in range(NPAIR):
                nc.scalar.activation(
                    e_sb[:, p, :], R_ps[:, 68 * p : 68 * p + 68], Exp,
                    bias=negm[:, p : p + 1], scale=1.0,
                )
            s1 = small.tile([128, 2], FP, tag="s1")
            nc.vector.reduce_sum(s1[:], e_sb[:], axis=X)
            rin = small.tile([128, 2], FP, tag="rin")
            nc.vector.reciprocal(rin[:], s1[:])
            outt = small.tile([128, 2, 68], FP, tag="outt")
            for p in range(NPAIR):
                # outt = (e * 1/den) * tril-mask, fused
                nc.vector.scalar_tensor_tensor(
                    outt[:, p, :],
                    e_sb[:, p, :],
                    rin[:, p : p + 1],
                    mm68[:, p, :],
                    op0=MULT,
                    op1=MULT,
                )
            nc.sync.dma_start(
                rout.rearrange("(p b) i c -> (b i) p c", p=2), outt[:, :, 1:66]
            )

    nc.compile()
    return nc


def _get_program(t_seq=T, enable_asserts=False):
    key = (t_seq, enable_asserts)
    if key not in _PROG_CACHE:
        _PROG_CACHE[key] = _build_program(t_seq, enable_asserts=enable_asserts)
    return _PROG_CACHE[key]


def _host_constants(core, q_pos_emb, k_pos_emb, Wsq, Wsk, chunk=CHUNK):
    """Single packed per-core constant tensor."""
    f32 = np.float32
    j = np.arange(64, dtype=np.float64)
    s = (1.0 / (chunk * j + 1.0)).astype(f32)  # per-bucket cumavg scale

    tri = np.triu(np.ones((64, 64), f32), k=1)  # [c, j] = 1 iff c < j
    tri_s = tri * s[None, :]
    lmat_s = np.zeros((128, 128), f32)
    lmat_s[0:64, 0:64] = tri_s
    lmat_s[64:128, 64:128] = tri_s
    idents = np.zeros((128, 128), f32)
    idents[np.arange(128), np.arange(128)] = np.concatenate([s, s])
    ident = np.eye(128, dtype=f32)

    rows = np.arange(64)[:, None]
    jj = np.arange(65)[None, :]
    # 68-wide per-pair block: cols 0/66/67 pad (NEG / 0), col 1+j for logit
    # j: valid iff j <= i (j=0 = pad row's constant zero logit), output
    # keeps j < i
    am = np.full((64, 68), NEG, f32)
    am[:, 1:66] = np.where(jj <= rows, 0.0, NEG)
    mm = np.zeros((64, 68), f32)
    mm[:, 1:66] = (jj < rows).astype(f32)
    am68 = np.concatenate([np.concatenate([am, am], axis=1)] * 2, axis=0)
    mm68 = np.concatenate([np.concatenate([mm, mm], axis=1)] * 2, axis=0)

    c128 = np.concatenate([lmat_s, idents, ident, am68, mm68], axis=1)

    wq_pt = np.zeros((2, 64, 104), f32)   # [pair][d][sq 0:40 | sk 64:104]
    wq_ft = np.zeros((2, 64, 104), f32)
    cblk = np.zeros((2, 104, 128), f32)   # [pair][skq-row][(b, j)]
    for p in range(NPAIR):
        for b in range(2):
            bh = core * BHC + 2 * p + b
            h = bh % HEADS
            r0 = 32 * b
            wq_pt[p, :, r0 : r0 + 8] = Wsq[0, h, 0:64, :]
            wq_pt[p, :, 64 + r0 : 64 + r0 + 8] = Wsk[0, h, 0:64, :]
            wq_ft[p, :, r0 : r0 + 8] = Wsq[0, h, 64:128, :]
            wq_ft[p, :, 64 + r0 : 64 + r0 + 8] = Wsk[0, h, 64:128, :]
            cq = q_pos_emb[0, h] @ Wsq[0, h, 128:192, :]  # (64, 8)
            ck = k_pos_emb[0, h] @ Wsk[0, h, 128:192, :]
            cblk[p, r0 : r0 + 8, 64 * b : 64 * b + 64] = cq.T
            cblk[p, 64 + r0 : 64 + r0 + 8, 64 * b : 64 * b + 64] = ck.T

    c64 = np.concatenate([wq_pt[0], wq_pt[1], wq_ft[0], wq_ft[1]], axis=1)
    c104 = np.concatenate([cblk[0], cblk[1]], axis=1)
    c104 = np.concatenate([c104, np.zeros((24, NC104), f32)], axis=0)
    cpack = np.concatenate([c128, c104], axis=1)
    assert cpack.shape == (128, NCALL), cpack.shape
    assert c64.shape == (64, NC64), c64.shape
    return {"cpack": cpack, "c64": c64}


def _run(k, q_pos_emb, k_pos_emb, Wsq, Wsk, trace=False, t_seq=T):
    nc = _get_program(t_seq)
    in_maps = []
    for core in range(NCORES):
        cm = _host_constants(
            core, q_pos_emb, k_pos_emb, Wsq, Wsk, chunk=t_seq // BUCKETS
        )
        cm["kin"] = np.ascontiguousarray(k[core * BHC : (core + 1) * BHC])
        in_maps.append(cm)
    res = bass_utils.run_bass_kernel_spmd(
        nc,
        in_maps,
        core_ids=list(range(NCORES)),
        trace=trace,
        **(TRACE_KWARGS if trace else {}),
    )
    global LAST_RESULTS
    LAST_RESULTS = res
    out = np.concatenate([r["rout"] for r in res.results], axis=0)
    return out, res


def kernel(**inputs):
    k = np.asarray(inputs["k"], np.float32)
    q_pos_emb = np.asarray(inputs["q_pos_emb"], np.float32)
    k_pos_emb = np.asarray(inputs["k_pos_emb"], np.float32)
    Wsq = np.asarray(inputs["Wsq"], np.float32)
    Wsk = np.asarray(inputs["Wsk"], np.float32)
    out, _ = _run(k, q_pos_emb, k_pos_emb, Wsq, Wsk, trace=TRACE)
    return out


# revision 14
# speedup vs baseline: 1.1233x; 1.1233x over previous
"""Trainium2 Bass kernel for CausalAttentionSortNet bucket-scoring.

Math (see reference): only `k` feeds the output. For each merged batch*head
slice, the cumulative-average of k is sampled at bucket starts (every 128th
row), which reduces to per-chunk sums + a strictly-triangular prefix matmul.
The rest is tiny per-bucket sort projections and a 64x65 masked softmax.

Sharding: data-parallel over the merged (batch*heads)=32 axis across 8 cores,
4 slices per core, processed as 2 pairs of 2 slices; a pair fills the
128-partition dim as partition=(slice_in_pair, chunk), free=(pair, row, dim)
so every partition's k data is contiguous 32KB HBM runs (DMA saturates all
16 engines at ~350 GB/s).

`q` (half of all input bytes) is never read by the reference computation, so
it is not even transferred to the device.

DMA-instruction budget: the hardware exposes ~12 DMA completion semaphores,
so a DMA instruction >=12 positions later reuses an earlier one's semaphore
and its issue blocks until that user completes. All constants ship in two
early-completing DMAs and the bulk tiles are uniform, so every reuse target
is long done by the time its semaphore is recycled (a version that put a
slow small-packet constant DMA in the reuse chain stalled the bulk queue
12us). Chunk first-rows are not a separate DMA: they arrive inside each
pair's first bulk tile, whose fold targets the tile's upper half so row 0
survives for the F-term matmuls.

Per-chunk reduction: each pair's rows stream as sub-tiles of
(16x7, 8, 4, 4) rows. Mid-stream, SBUF port contention caps DVE at
~1.8ns/elem and GpSimd at ~2.6ns/elem (vs 1.04/2.0 idle), so each sub-tile
gets an INDEPENDENT halving-fold chain (contiguous tensor_adds down to one
row -> its own partial-sum slot) and the chains are statically balanced
across both engines; a long serial cascade on one engine trailed the
stream by 12us. The PE (otherwise idle) folds every partial into the
scaled-prefix via one matmul per sub-tile against the tril*scale constant,
accumulating in that pair's PSUM bank, opened by the F*diag(s) seed and
closed by the last sub-tile's matmul. Small sub-tiles stream last so the
post-stream tail is two ~0.3us fold chains plus the epilogue.
"""

from contextlib import ExitStack

import numpy as np

import concourse.bacc as bacc
import concourse.mybir as mybir
import concourse.tile as tile
from concourse import bass_utils

# Problem constants (hardcoded per contract; kernel.py must be self-contained).
B, HEADS, BUCKETS, DIM, DIM_SORT, T = 4, 8, 64, 64, 8, 8192
BH = B * HEADS            # 32 merged batch*head slices
NCORES = 8
BHC = BH // NCORES        # 4 slices per core
NPAIR = BHC // 2          # 2 pairs per core
CHUNK = T // BUCKETS      # 128 rows per bucket
NEG = -1.0e30             # softmax mask value (underflows exp to exactly 0)
FP = mybir.dt.float32

# packed-constant column offsets
NC128 = 128 * 3 + 136 * 2
NC64 = 4 * 104
NC104 = 2 * 128
NCALL = NC128 + NC104

# pair-1's split-tile fold chains run on GpSimd; pair-0's and the merged
# tail tiles on the faster DVE

TRACE = False  # set by test.py for profiling runs
TRACE_KWARGS = {}  # extra run_bass_kernel_spmd kwargs for profiling runs
LAST_RESULTS = None  # BassKernelResults of the most recent run

_PROG_CACHE = {}


# per-pair sub-tiles (fold-chain per tile): big ones first for per-op
# overhead amortization, then pair-MERGED small tail tiles (one tile holds
# both pairs, halving tail op count)
SPLIT_SIZES = (32, 32, 32, 16)
MERGED_SIZES = (8, 4, 4)
assert sum(SPLIT_SIZES) + sum(MERGED_SIZES) == CHUNK


def _build_program(t_seq=T, enable_asserts=False, debug_taps=False):
    chunk = t_seq // BUCKETS
    assert chunk == CHUNK, "sub-tile schedule is tuned for chunk=128"
    nsplit = len(SPLIT_SIZES)
    nsub = nsplit + len(MERGED_SIZES)

    nc = bacc.Bacc(
        "TRN2",
        target_bir_lowering=False,
        debug=False,
        enable_asserts=enable_asserts,
        num_devices=NCORES,
    )

    def din(name, shape):
        return nc.dram_tensor(name, shape, FP, kind="ExternalInput").ap()

    kin = din("kin", (BHC, t_seq, DIM))
    # packed constants, two DMAs:
    # cpack cols 0:656     [lmat_s | idents | ident | am68 | mm68]
    # cpack cols 644:900   c104 = per pair (104, 128) cq/ck blocks (rows 104:128 zero)
    # c64 (64, 416)        [wqk_pt_p0 | wqk_pt_p1 | wqk_ft_p0 | wqk_ft_p1]
    cpack = din("cpack", (128, NCALL))
    c64 = din("c64", (64, NC64))
    rout = nc.dram_tensor(
        "rout", (BHC, BUCKETS, BUCKETS + 1), FP, kind="ExternalOutput"
    ).ap()
    taps = {}
    if debug_taps:
        taps["pt"] = nc.dram_tensor("tap_pt", (128, 128), FP, kind="ExternalOutput").ap()
        taps["ft"] = nc.dram_tensor("tap_ft", (128, 128), FP, kind="ExternalOutput").ap()
        taps["par"] = nc.dram_tensor("tap_par", (128, 128), FP, kind="ExternalOutput").ap()

    X = mybir.AxisListType.X
    Exp = mybir.ActivationFunctionType.Exp
    MULT = mybir.AluOpType.mult

    with tile.TileContext(nc) as tc:
        with ExitStack() as ctx:
            singles = ctx.enter_context(tc.tile_pool(name="singles", bufs=1))
            kpools = [
                ctx.enter_context(
                    tc.tile_pool(name=f"kpool{s}", bufs=2 if s < nsplit else 1)
                )
                for s in range(nsub)
            ]
            parp = ctx.enter_context(tc.tile_pool(name="parp", bufs=nsub))
            small = ctx.enter_context(tc.tile_pool(name="small", bufs=2))
            pp = ctx.enter_context(tc.tile_pool(name="pp", bufs=1, space="PSUM"))

            cp_sb = singles.tile([128, NCALL], FP, tag="cpack")
            nc.scalar.dma_start(cp_sb[:], cpack)
            c64_sb = singles.tile([64, NC64], FP, tag="c64")
            nc.scalar.dma_start(c64_sb[:], c64)

            # ---- bulk k sub-tile DMAs, single queue, pair 1 leading so
            # its GpSimd chains start first (contiguous rows*256B runs per
            # partition)
            ksrcs = [
                kin[2 * p : 2 * p + 2].rearrange("b (c r) d -> (b c) r d", r=chunk)
                for p in range(NPAIR)
            ]
            kmerged = kin.rearrange(
                "(p b) (c r) d -> (b c) p r d", p=NPAIR, r=chunk
            )
            kts = {}
            r0 = 0
            for s, rs in enumerate(SPLIT_SIZES):
                for p in (1, 0):
                    kt = kpools[s].tile(
                        [128, rs, DIM], FP, tag=f"kt{s}", name=f"kt{s}_{p}"
                    )
                    nc.sync.dma_start(kt[:], ksrcs[p][:, r0 : r0 + rs, :])
                    kts[(p, s)] = kt
                r0 += rs
            for i, rs in enumerate(MERGED_SIZES):
                s = nsplit + i
                kt = kpools[s].tile(
                    [128, NPAIR, rs, DIM], FP, tag=f"kt{s}", name=f"kt{s}m"
                )
                nc.sync.dma_start(kt[:], kmerged[:, :, r0 : r0 + rs, :])
                kts[s] = kt
                r0 += rs

            lmat_s = cp_sb[:, 0:128]
            idents = cp_sb[:, 128:256]
            ident = cp_sb[:, 256:384]
            # am68 (cols 384:520) is consumed directly by the R-group matmul
            mm68 = cp_sb[:, 520:656].rearrange("q (p j) -> q p j", p=2)

            # ---- PSUM groups, one bank per (pair, tensor): FT_p is F
            # transposed; PT_p is opened by the F*diag(s) seed and closed by
            # that pair's chunk-sum prefix matmul. F = row 0 of the first
            # bulk tile. Separate banks let pair 0's whole epilogue run while
            # pair 1 is still streaming.
            # full-partition PSUM tiles: a 64-partition tile can be packed
            # at partition offset 64 of another tile's bank, and its group's
            # deferred-zero bookkeeping then poisons that tile's rows 64:128
            PT_ps = [
                pp.tile([128, 128], FP, tag=f"PT{p}", name=f"PT_ps{p}")
                for p in range(NPAIR)
            ]
            FT_ps = [
                pp.tile([128, 128], FP, tag=f"FT{p}", name=f"FT_ps{p}")
                for p in range(NPAIR)
            ]
            for p in range(NPAIR):
                nc.tensor.matmul(
                    FT_ps[p][0:64, :],
                    lhsT=kts[(p, 0)][:, 0, :],
                    rhs=ident,
                    start=True,
                    stop=True,
                )
                nc.tensor.matmul(
                    PT_ps[p][0:64, :],
                    lhsT=kts[(p, 0)][:, 0, :],
                    rhs=idents,
                    start=True,
                    stop=False,
                )

            # ---- per-sub-tile fold chains + per-sub-tile prefix matmuls.
            # Each chain folds its tile down to one row (first fold targets
            # the upper half so row 0, the F term, survives in tile 0) into
            # its own partial-sum slot; the PE folds every partial into the
            # pair's scaled-prefix PSUM bank as it appears.
            pars = [
                parp.tile([128, NPAIR, DIM], FP, tag=f"par{s}", name=f"par{s}")
                for s in range(nsub)
            ]

            def fold_chain(e, view, viewr, rs, par_dst):
                # view(a, b) slices rows [a, b); viewr(r) indexes one row;
                # row 0 is preserved (first fold targets the upper half)
                h = rs // 2
                e.tensor_add(view(h, rs), view(h, rs), view(0, h))
                lo, xr = h, h
                while xr > 2:
                    hh = xr // 2
                    e.tensor_add(
                        view(lo, lo + hh), view(lo, lo + hh),
                        view(lo + hh, lo + xr),
                    )
                    xr = hh
                e.tensor_add(par_dst, viewr(lo), viewr(lo + 1))

            for s, rs in enumerate(SPLIT_SIZES):
                for p in (1, 0):
                    t = kts[(p, s)]
                    e = nc.gpsimd if p == 1 else nc.vector
                    fold_chain(
                        e,
                        lambda a, b, t=t: t[:, a:b, :],
                        lambda r, t=t: t[:, r, :],
                        rs,
                        pars[s][:, p, :],
                    )
                    nc.tensor.matmul(
                        PT_ps[p][0:64, :],
                        lhsT=pars[s][:, p, :],
                        rhs=lmat_s,
                        start=False,
                        stop=False,
                    )
            for i, rs in enumerate(MERGED_SIZES):
                s = nsplit + i
                t = kts[s]
                fold_chain(
                    nc.vector,
                    lambda a, b, t=t: t[:, :, a:b, :],
                    lambda r, t=t: t[:, :, r, :],
                    rs,
                    pars[s][:, :, :],
                )
                for p in (1, 0):
                    nc.tensor.matmul(
                        PT_ps[p][0:64, :],
                        lhsT=pars[s][:, p, :],
                        rhs=lmat_s,
                        start=False,
                        stop=s == nsub - 1,
                    )

            # ---- sort projections (per pair), batched softmax (both pairs)
            PT_sb = [
                small.tile([64, 128], FP, tag=f"PTs{p}", name=f"PT_sb{p}")
                for p in range(NPAIR)
            ]
            FT_sb = [
                small.tile([64, 128], FP, tag=f"FTs{p}", name=f"FT_sb{p}")
                for p in range(NPAIR)
            ]
            for p in range(NPAIR):
                nc.scalar.copy(FT_sb[p][:], FT_ps[p][0:64, :])
                nc.scalar.copy(PT_sb[p][:], PT_ps[p][0:64, :])
            if debug_taps:
                for p in range(NPAIR):
                    nc.sync.dma_start(taps["pt"][64 * p : 64 * p + 64], PT_sb[p][:])
                    nc.sync.dma_start(taps["ft"][64 * p : 64 * p + 64], FT_sb[p][:])


            # SKQ rows: 0:40 sort-q blocks (b0 at 0:8, b1 at 32:40),
            #           64:104 sort-k blocks (b0 at 64:72, b1 at 96:104);
            # one PSUM bank per pair: each holds a long-open accumulation group
            # opened by the constant-term matmul (ready at kernel start) and
            # closed by the PT-part matmul (the only one on the critical tail)
            C104O = NC128
            SQs = []
            RKs = []
            for p in range(NPAIR):
                sk_ps_t = pp.tile([128, 128], FP, tag=f"SKQ{p}", name=f"skq{p}")
                sk_ps = sk_ps_t[0:104, :]
                nc.tensor.matmul(
                    sk_ps,
                    lhsT=ident[0:104, 0:104],
                    rhs=cp_sb[0:104, C104O + 128 * p : C104O + 128 * p + 128],
                    start=True,
                    stop=False,
                    skip_group_check=True,
                )
                nc.tensor.matmul(
                    sk_ps,
                    lhsT=c64_sb[:, 208 + 104 * p : 312 + 104 * p],
                    rhs=FT_sb[p][:],
                    start=False,
                    stop=False,
                    skip_group_check=True,
                )
                nc.tensor.matmul(
                    sk_ps,
                    lhsT=c64_sb[:, 104 * p : 104 * p + 104],
                    rhs=PT_sb[p][:],
                    start=False,
                    stop=True,
                    skip_group_check=True,
                )
                sq_sb = small.tile([40, 128], FP, tag=f"SQ{p}")
                nc.scalar.copy(sq_sb[:], sk_ps[0:40, :])
                rk_sb = small.tile([40, 128], FP, tag=f"RK{p}")
                nc.vector.tensor_copy(rk_sb[:], sk_ps[64:104, :])
                SQs.append(sq_sb)
                RKs.append(rk_sb)

            # R group, 65 logit columns per pair: opened early by an
            # identity-weighted matmul that seeds the bank with the additive
            # causal mask (whose column 0 is the pad-row's constant zero
            # logit); the four sq.sk matmuls then accumulate into columns
            # 1:65 of their quadrants, so the masked logits sit in PSUM with
            # no extra elementwise pass and no separate zero-column handling
            # per-pair 68-wide blocks: col 0 pad, col 1 the pad-row's
            # constant zero logit (both written only by the mask seed), cols
            # 2:66 the sq.sk logits, 66:68 pad. The 68 width keeps the
            # partition-64 quadrant writes inside the seed-cleared PSUM
            # zero-region window (64 * width-bytes must be 0 mod 2048) and
            # their byte offsets 8-aligned.
            R_ps = pp.tile([128, 2 * 68], FP, tag="R")
            nc.tensor.matmul(
                R_ps[:],
                lhsT=ident,
                rhs=cp_sb[:, 384:520],
                start=True,
                stop=False,
                skip_group_check=True,
            )
            for p in range(NPAIR):
                nc.tensor.matmul(
                    R_ps[0:64, 68 * p + 2 : 68 * p + 66],
                    lhsT=SQs[p][0:8, 0:64],
                    rhs=RKs[p][0:8, 0:64],
                    start=False,
                    stop=False,
                    skip_group_check=True,
                )
                nc.tensor.matmul(
                    R_ps[64:128, 68 * p + 2 : 68 * p + 66],
                    lhsT=SQs[p][32:40, 64:128],
                    rhs=RKs[p][32:40, 64:128],
                    start=False,
                    stop=p == NPAIR - 1,
                    skip_group_check=True,
                )

            # masked softmax over the 65 logits, both pairs batched along
            # the free axis: cols = (pair, j). Column 0 (the zero logit) is
            # already in R, so max/sum/mask need no special-casing.
            Rm = R_ps[:].rearrange("q (p j) -> q p j", p=2)
            mx = small.tile([128, 2], FP, tag="mx")
            nc.vector.reduce_max(mx[:], Rm, axis=X)
            negm = small.tile([128, 2], FP, tag="negm")
            nc.vector.tensor_scalar(
                negm[:], mx[:], 0.0, -1.0,
                op0=mybir.AluOpType.max, op1=MULT,
            )
            e_sb = small.tile([128, 2, 68], FP, tag="e")
            for p in range(NPAIR):
                nc.scalar.activation(
                    e_sb[:, p, :], R_ps[:, 68 * p : 68 * p + 68], Exp,
                    bias=negm[:, p : p + 1], scale=1.0,
                )
            s1 = small.tile([128, 2], FP, tag="s1")
            nc.vector.reduce_sum(s1[:], e_sb[:], axis=X)
            rin = small.tile([128, 2], FP, tag="rin")
            nc.vector.reciprocal(rin[:], s1[:])
            outt = small.tile([128, 2, 68], FP, tag="outt")
            for p in range(NPAIR):
                # outt = (e * 1/den) * tril-mask, fused
                nc.vector.scalar_tensor_tensor(
                    outt[:, p, :],
                    e_sb[:, p, :],
                    rin[:, p : p + 1],
                    mm68[:, p, :],
                    op0=MULT,
                    op1=MULT,
                )
            nc.sync.dma_start(
                rout.rearrange("(p b) i c -> (b i) p c", p=2), outt[:, :, 1:66]
            )

    nc.compile()
    return nc


def _get_program(t_seq=T, enable_asserts=False):
    key = (t_seq, enable_asserts)
    if key not in _PROG_CACHE:
        _PROG_CACHE[key] = _build_program(t_seq, enable_asserts=enable_asserts)
    return _PROG_CACHE[key]


def _host_constants(core, q_pos_emb, k_pos_emb, Wsq, Wsk, chunk=CHUNK):
    """Single packed per-core constant tensor."""
    f32 = np.float32
    j = np.arange(64, dtype=np.float64)
    s = (1.0 / (chunk * j + 1.0)).astype(f32)  # per-bucket cumavg scale

    tri = np.triu(np.ones((64, 64), f32), k=1)  # [c, j] = 1 iff c < j
    tri_s = tri * s[None, :]
    lmat_s = np.zeros((128, 128), f32)
    lmat_s[0:64, 0:64] = tri_s
    lmat_s[64:128, 64:128] = tri_s
    idents = np.zeros((128, 128), f32)
    idents[np.arange(128), np.arange(128)] = np.concatenate([s, s])
    ident = np.eye(128, dtype=f32)

    rows = np.arange(64)[:, None]
    jj = np.arange(65)[None, :]
    # 68-wide per-pair block: cols 0/66/67 pad (NEG / 0), col 1+j for logit
    # j: valid iff j <= i (j=0 = pad row's constant zero logit), output
    # keeps j < i
    am = np.full((64, 68), NEG, f32)
    am[:, 1:66] = np.where(jj <= rows, 0.0, NEG)
    mm = np.zeros((64, 68), f32)
    mm[:, 1:66] = (jj < rows).astype(f32)
    am68 = np.concatenate([np.concatenate([am, am], axis=1)] * 2, axis=0)
    mm68 = np.concatenate([np.concatenate([mm, mm], axis=1)] * 2, axis=0)

    c128 = np.concatenate([lmat_s, idents, ident, am68, mm68], axis=1)

    wq_pt = np.zeros((2, 64, 104), f32)   # [pair][d][sq 0:40 | sk 64:104]
    wq_ft = np.zeros((2, 64, 104), f32)
    cblk = np.zeros((2, 104, 128), f32)   # [pair][skq-row][(b, j)]
    for p in range(NPAIR):
        for b in range(2):
            bh = core * BHC + 2 * p + b
            h = bh % HEADS
            r0 = 32 * b
            wq_pt[p, :, r0 : r0 + 8] = Wsq[0, h, 0:64, :]
            wq_pt[p, :, 64 + r0 : 64 + r0 + 8] = Wsk[0, h, 0:64, :]
            wq_ft[p, :, r0 : r0 + 8] = Wsq[0, h, 64:128, :]
            wq_ft[p, :, 64 + r0 : 64 + r0 + 8] = Wsk[0, h, 64:128, :]
            cq = q_pos_emb[0, h] @ Wsq[0, h, 128:192, :]  # (64, 8)
            ck = k_pos_emb[0, h] @ Wsk[0, h, 128:192, :]
            cblk[p, r0 : r0 + 8, 64 * b : 64 * b + 64] = cq.T
            cblk[p, 64 + r0 : 64 + r0 + 8, 64 * b : 64 * b + 64] = ck.T

    c64 = np.concatenate([wq_pt[0], wq_pt[1], wq_ft[0], wq_ft[1]], axis=1)
    c104 = np.concatenate([cblk[0], cblk[1]], axis=1)
    c104 = np.concatenate([c104, np.zeros((24, NC104), f32)], axis=0)
    cpack = np.concatenate([c128, c104], axis=1)
    assert cpack.shape == (128, NCALL), cpack.shape
    assert c64.shape == (64, NC64), c64.shape
    return {"cpack": cpack, "c64": c64}


def _run(k, q_pos_emb, k_pos_emb, Wsq, Wsk, trace=False, t_seq=T):
    nc = _get_program(t_seq)
    in_maps = []
    for core in range(NCORES):
        cm = _host_constants(
            core, q_pos_emb, k_pos_emb, Wsq, Wsk, chunk=t_seq // BUCKETS
        )
        cm["kin"] = np.ascontiguousarray(k[core * BHC : (core + 1) * BHC])
        in_maps.append(cm)
    res = bass_utils.run_bass_kernel_spmd(
        nc,
        in_maps,
        core_ids=list(range(NCORES)),
        trace=trace,
        **(TRACE_KWARGS if trace else {}),
    )
    global LAST_RESULTS
    LAST_RESULTS = res
    out = np.concatenate([r["rout"] for r in res.results], axis=0)
    return out, res


def kernel(**inputs):
    k = np.asarray(inputs["k"], np.float32)
    q_pos_emb = np.asarray(inputs["q_pos_emb"], np.float32)
    k_pos_emb = np.asarray(inputs["k_pos_emb"], np.float32)
    Wsq = np.asarray(inputs["Wsq"], np.float32)
    Wsk = np.asarray(inputs["Wsk"], np.float32)
    out, _ = _run(k, q_pos_emb, k_pos_emb, Wsq, Wsk, trace=TRACE)
    return out
